# revision 6
# baseline (speedup 1.0000x reference)
"""CrossMultiheadDiffAttn Trainium2 kernel (v3).

Same sharding as v2: 8 cores = 2 batches x 4 head-groups.
New options vs v2:
  pretrans: host pre-transposes x inputs to [E, T]; straight DMA loads
            (no XBAR dma_start_transpose).
  mergez:   softmax denominator column merged into the AV matmul rhs
            ([vv | ones] 65-wide slots in 3 PSUM bank tiles) instead of
            2048 separate 1-column matmuls (halves PE weight loads).
  unroll:   emit the whole body N times in one NEFF (for timing: amortizes
            the per-call RPC floor so device exec becomes measurable).
"""

import math

import numpy as np

import concourse.bacc as bacc
import concourse.bass as bass
import concourse.mybir as mybir
import concourse.tile as tile
from concourse import masks
from concourse.bass_utils import run_bass_kernel_spmd
from contextlib import ExitStack

F32 = mybir.dt.float32
F16 = mybir.dt.float16
I16 = mybir.dt.int16
I32 = mybir.dt.int32
AF = mybir.ActivationFunctionType
OP = mybir.AluOpType

EMBED = 1024
HEADS = 16
DEPTH = 12
D = 32
LAMBDA_INIT = 0.8 - 0.6 * math.exp(-0.3 * DEPTH)
T = 2048
S = 2048
HC = 256
NE = EMBED // 128
NS = S // 128
PAIRS = 4
EPS = 1e-5

ACOL = 512
SCH_A = 1024.0 / math.log(2.0)
SCH_B = 15360.0 - 47.0

_cache = {}


def _emit(nc, tc, ctx, dram, lam, phases, it):
    """Emit one full iteration of the kernel body. `it` suffixes pool names."""
    xq_d, xkv_d, wq_d, wk_d, wv_d, wo_d, out_d = dram
    pretrans = xq_d.shape[0] == EMBED

    persist = ctx.enter_context(tc.tile_pool(name=f"persist{it}", bufs=1))
    ident = persist.tile([128, 128], F16, tag="ident", name="ident")
    masks.make_identity(nc, ident[:])
    ones16 = persist.tile([128, 1], F16, tag="ones", name="ones")
    nc.vector.memset(ones16[:], 1.0)
    nlam16 = persist.tile([128, 1], F16, tag="nlam", name="nlam")
    nc.vector.memset(nlam16[:], -1.0 / lam)

    qt = [persist.tile([128, T], F16, tag=f"qt{m}", name=f"qt{m}")
          for m in range(2)]
    kt = [persist.tile([128, S], F16, tag=f"kt{m}", name=f"kt{m}")
          for m in range(2)]
    # vv tiles: [vv(64 per pair) | ones] 65-wide groups when mergez
    vw = 65 if MERGEZ else 64
    vv = [persist.tile([128, 4 * vw], F16, tag=f"vv{s}", name=f"vv{s}")
          for s in range(NS)]
    at = [persist.tile([128, T], F16, tag=f"at{c}", name=f"at{c}")
          for c in range(2)]
    wo_sb = [persist.tile([128, EMBED], F16, tag=f"wo{c}", name=f"wo{c}")
             for c in range(2)]

    # ---------------- Phase A: loads + projections ----------------
    with ExitStack() as actx:
        apool = actx.enter_context(tc.tile_pool(name=f"ain{it}", bufs=1))
        xkv = [apool.tile([128, S], F16, tag=f"xkv{e}", name=f"xkv{e}")
               for e in range(NE)]
        xq = [apool.tile([128, T], F16, tag=f"xq{e}", name=f"xq{e}")
              for e in range(NE)]
        wk_sb = [apool.tile([128, HC], F16, tag=f"wk{e}", name=f"wk{e}")
                 for e in range(NE)]
        wv_sb = [apool.tile([128, HC], F16, tag=f"wv{e}", name=f"wv{e}")
                 for e in range(NE)]
        wq_sb = [apool.tile([128, HC], F16, tag=f"wq{e}", name=f"wq{e}")
                 for e in range(NE)]

        if pretrans:
            nc.sync.dma_start(out=xkv[0][:], in_=xkv_d[0:128, :])
        for e in range(NE):
            nc.sync.dma_start(out=wv_sb[e][:], in_=wv_d[e * 128:(e + 1) * 128, :])
        for e in range(NE):
            if pretrans:
                if e > 0:
                    nc.sync.dma_start(out=xkv[e][:],
                                      in_=xkv_d[e * 128:(e + 1) * 128, :])
            else:
                nc.sync.dma_start_transpose(
                    out=xkv[e][:], in_=xkv_d[:, e * 128:(e + 1) * 128])
        for e in range(NE):
            nc.sync.dma_start(out=wk_sb[e][:], in_=wk_d[e * 128:(e + 1) * 128, :])
            nc.sync.dma_start(out=wq_sb[e][:], in_=wq_d[e * 128:(e + 1) * 128, :])
        for e in range(NE):
            if pretrans:
                nc.sync.dma_start(out=xq[e][:],
                                  in_=xq_d[e * 128:(e + 1) * 128, :])
            else:
                nc.sync.dma_start_transpose(
                    out=xq[e][:], in_=xq_d[:, e * 128:(e + 1) * 128])
        for c in range(2):
            nc.sync.dma_start(out=wo_sb[c][:], in_=wo_d[c * 128:(c + 1) * 128, :])

        with tc.tile_pool(name=f"vpsum{it}", bufs=1, space="PSUM") as vpool:
            vps = [vpool.tile([128, 512], F32, tag=f"vps{j}",
                              name=f"vps{j}") for j in range(8)]
            for e in range(NE):
                for s in range(NS):
                    j, half = divmod(s, 2)
                    nc.tensor.matmul(
                        vps[j][:, half * HC:(half + 1) * HC],
                        lhsT=xkv[e][:, s * 128:(s + 1) * 128],
                        rhs=wv_sb[e][:],
                        start=(e == 0 and half == 0),
                        stop=(e == NE - 1 and half == 1),
                        skip_group_check=True)
            for s in range(NS):
                j, half = divmod(s, 2)
                if MERGEZ:
                    for p in range(PAIRS):
                        nc.scalar.copy(
                            vv[s][:, p * 65:p * 65 + 64],
                            vps[j][:, half * HC + p * 64:half * HC + (p + 1) * 64])
                    nc.vector.memset(vv[s][:, 64:4 * 65:65], 1.0)
                else:
                    nc.scalar.copy(vv[s][:], vps[j][:, half * HC:(half + 1) * HC])

        ppool = actx.enter_context(
            tc.tile_pool(name=f"ppsum{it}", bufs=4, space="PSUM"))

        for w_sb, x_sb, dst in ((wk_sb, xkv, kt), (wq_sb, xq, qt)):
            for m in range(2):
                for n in range(4):
                    ps = ppool.tile([128, 512], F32, tag="pj",
                                    name=f"qkps{m}_{n}")
                    for e in range(NE):
                        nc.tensor.matmul(
                            ps[:],
                            lhsT=w_sb[e][:, m * 128:(m + 1) * 128],
                            rhs=x_sb[e][:, n * 512:(n + 1) * 512],
                            start=(e == 0), stop=(e == NE - 1))
                    nc.vector.tensor_copy(dst[m][:, n * 512:(n + 1) * 512],
                                          ps[:])

    if "B" not in phases:
        for m in range(2):
            st = persist.tile([128, T], F16, tag=f"dbg{m}", name=f"dbg{m}")
            nc.vector.tensor_copy(st[:], qt[m][:])
            nc.sync.dma_start(out=out_d[m * 128:(m + 1) * 128, :], in_=st[:])
            st2 = persist.tile([128, T], F16, tag=f"dbg2{m}", name=f"dbg2{m}")
            nc.vector.tensor_copy(st2[:], kt[m][:])
            nc.sync.dma_start(out=out_d[(2 + m) * 128:(3 + m) * 128, :],
                              in_=st2[:])

    # ---------------- Phase B: attention ----------------
    with ExitStack() as bctx:
      if "B" in phases:
        scp = bctx.enter_context(
            tc.tile_pool(name=f"scp{it}", bufs=2, space="PSUM"))
        avp = bctx.enter_context(
            tc.tile_pool(name=f"avp{it}", bufs=1, space="PSUM"))
        if not MERGEZ:
            ztp = bctx.enter_context(
                tc.tile_pool(name=f"ztp{it}", bufs=1, space="PSUM"))
        tpp = bctx.enter_context(
            tc.tile_pool(name=f"tpp{it}", bufs=1, space="PSUM"))
        ptp = bctx.enter_context(tc.tile_pool(name=f"ptp{it}", bufs=5))
        epp = bctx.enter_context(tc.tile_pool(name=f"epp{it}", bufs=3))

        pending = []

        for p in range(PAIRS):
            m = p // 2
            row = (p % 2) * 64
            for th in range(2):
                if MERGEZ:
                    # 16 slots of 65 (8 t-chunks x 2 hi), 6 per bank tile
                    avb = [avp.tile([128, 512], F32, tag=f"avb{j}",
                                    name=f"avb{j}_{p}_{th}") for j in range(3)]

                    def av_slot(hi, t, avb=avb):
                        k = hi * 8 + t
                        j, kk = divmod(k, 6)
                        return avb[j][:, kk * 65:(kk + 1) * 65]
                else:
                    av = avp.tile([128, 1024], F32, tag="av",
                                  name=f"av{p}_{th}")
                    zt = ztp.tile([128, 16], F32, tag="zt", name=f"zt{p}_{th}")
                units = [(s, hi) for s in range(NS) for hi in range(2)]
                sc_t, pt_t = {}, {}

                def emit_sc_exp(u, p=p, th=th, m=m, sc_t=sc_t, pt_t=pt_t):
                    s, hi = u
                    h = 2 * p + hi
                    qo = (h % 4) * 32
                    sca = scp.tile([128, 512], F32, tag="sca",
                                   name=f"sca{p}_{th}_{s}_{hi}")
                    scd = scp.tile([128, 512], F32, tag="scd",
                                   name=f"scd{p}_{th}_{s}_{hi}")
                    for nj, sc in ((0, sca), (1, scd)):
                        nc.tensor.matmul(
                            sc[:],
                            lhsT=kt[m][qo:qo + 32, s * 128:(s + 1) * 128],
                            rhs=qt[m][qo:qo + 32,
                                      th * 1024 + nj * 512:
                                      th * 1024 + (nj + 1) * 512],
                            start=True, stop=True, tile_position=(qo, 0))
                    pta = ptp.tile([128, ACOL], F16, tag="pta",
                                   name=f"pta{p}_{th}_{s}_{hi}")
                    ptd = ptp.tile([128, 1024 - ACOL], I16, tag="ptd",
                                   name=f"ptd{p}_{th}_{s}_{hi}")
                    nc.scalar.activation(pta[:], sca[:], AF.Exp)
                    nc.vector.tensor_scalar(
                        ptd[:], scd[:],
                        SCH_A, SCH_B, OP.mult, OP.add)
                    sc_t[u] = (sca, scd)
                    pt_t[u] = (pta, ptd)

                nsplit = ACOL // 128

                def pt_chunk(pt, t):
                    pta, ptd = pt
                    if t < nsplit:
                        return pta[:, t * 128:(t + 1) * 128]
                    return ptd[:, (t - nsplit) * 128:
                               (t - nsplit + 1) * 128].bitcast(F16)

                if MERGEZ:
                    def emit_av(u, p=p, th=th, sc_t=sc_t, pt_t=pt_t,
                                av_slot=av_slot):
                        s, hi = u
                        pt = pt_t.pop(u)
                        del sc_t[u]
                        for t in range(8):
                            k = hi * 8 + t
                            nc.tensor.matmul(
                                av_slot(hi, t),
                                lhsT=pt_chunk(pt, t),
                                rhs=vv[s][:, p * 65:(p + 1) * 65],
                                start=(s == 0 and k in (0, 6, 12)),
                                stop=(s == NS - 1 and k in (5, 11, 15)),
                                skip_group_check=True)
                else:
                    def emit_av(u, p=p, av=av, zt=zt, sc_t=sc_t, pt_t=pt_t):
                        s, hi = u
                        pt = pt_t.pop(u)
                        del sc_t[u]
                        for t in range(8):
                            nc.tensor.matmul(
                                av[:, hi * 512 + t * 64:hi * 512 + (t + 1) * 64],
                                lhsT=pt_chunk(pt, t),
                                rhs=vv[s][:, p * 64:(p + 1) * 64],
                                start=(s == 0 and t == 0),
                                stop=(s == NS - 1 and t == 7),
                                skip_group_check=True)
                        for t in range(8):
                            nc.tensor.matmul(
                                zt[:, hi * 8 + t:hi * 8 + t + 1],
                                lhsT=pt_chunk(pt, t),
                                rhs=ones16[:],
                                start=(s == 0 and hi == 0 and t == 0),
                                stop=(s == NS - 1 and hi == 1 and t == 7),
                                skip_group_check=True)

                emit_sc_exp(units[0])
                for i, u in enumerate(units):
                    if i + 1 < len(units):
                        emit_sc_exp(units[i + 1])
                    emit_av(u)
                    while pending and pending[0][0] <= i:
                        pending.pop(0)[1]()

                # ---- epilogue ----
                rec = epp.tile([128, 16], F32, tag="rec",
                               name=f"rec{p}_{th}")
                if MERGEZ:
                    # gather z cols (slot col 64, stride 65) from 3 bank tiles
                    nc.vector.reciprocal(
                        rec[:, 0:6], avb[0][:, 64:64 + 6 * 65:65])
                    nc.vector.reciprocal(
                        rec[:, 6:12], avb[1][:, 64:64 + 6 * 65:65])
                    nc.vector.reciprocal(
                        rec[:, 12:16], avb[2][:, 64:64 + 4 * 65:65])
                else:
                    nc.vector.reciprocal(rec[:], zt[:, 0:16])
                rec2l = epp.tile([128, 8], F32, tag="rec2l",
                                 name=f"rec2l{p}_{th}")
                nc.vector.tensor_scalar_mul(rec2l[:], rec[:, 8:16], -lam)

                tmp = epp.tile([128, 512], F32, tag="tmp",
                               name=f"tmp{p}_{th}")
                e2 = epp.tile([128, 512], F32, tag="e2", name=f"e2{p}_{th}")
                if MERGEZ:
                    # slots k=0..15 (65-wide, 6|6|4 per bank tile); o-part is
                    # a 3D strided view, scaled by the per-partition rec col.
                    def oview(j, a, n):
                        return avb[j][:, a * 65:(a + n) * 65].rearrange(
                            "p (t c) -> p t c", c=65)[:, :, 0:64]
                    def rview(r, a, n):
                        return r[:, a:a + n].unsqueeze(2).broadcast_to(
                            [128, n, 64])
                    t3 = tmp[:].rearrange("p (t c) -> p t c", c=64)
                    e3 = e2[:].rearrange("p (t c) -> p t c", c=64)
                    nc.vector.tensor_mul(t3[:, 0:6], oview(0, 0, 6),
                                         rview(rec, 0, 6))
                    nc.vector.tensor_mul(t3[:, 6:8], oview(1, 0, 2),
                                         rview(rec, 6, 2))
                    nc.vector.tensor_mul(e3[:, 0:4], oview(1, 2, 4),
                                         rview(rec2l, 0, 4))
                    nc.vector.tensor_mul(e3[:, 4:8], oview(2, 0, 4),
                                         rview(rec2l, 4, 4))
                else:
                    o1 = av[:, 0:512].rearrange("p (t c) -> p t c", c=64)
                    o2 = av[:, 512:1024].rearrange("p (t c) -> p t c", c=64)
                    recB = rec[:, 0:8].unsqueeze(2).broadcast_to([128, 8, 64])
                    rec2B = rec2l[:].unsqueeze(2).broadcast_to([128, 8, 64])
                    nc.vector.tensor_mul(
                        tmp[:].rearrange("p (t c) -> p t c", c=64), o1, recB)
                    nc.vector.tensor_mul(
                        e2[:].rearrange("p (t c) -> p t c", c=64), o2, rec2B)

                def late1(tmp=tmp, e2=e2, p=p, th=th):
                    attn = epp.tile([128, 512], F32, tag="attn",
                                    name=f"attn{p}_{th}")
                    nc.gpsimd.tensor_add(attn[:], tmp[:], e2[:])
                    sq = epp.tile([128, 512], F32, tag="sq",
                                  name=f"sq{p}_{th}")
                    nc.gpsimd.tensor_mul(sq[:], attn[:], attn[:])
                    ssq = epp.tile([128, 8], F32, tag="ssq",
                                   name=f"ssq{p}_{th}")
                    nc.vector.reduce_sum(
                        ssq[:], sq[:].rearrange("p (t c) -> p t c", c=64),
                        axis=mybir.AxisListType.X)
                    mm_ = epp.tile([128, 8], F32, tag="mm", name=f"mm{p}_{th}")
                    nc.vector.tensor_scalar(mm_[:], ssq[:], 1.0 / 64.0,
                                            EPS, OP.mult, OP.add)
                    it_ = epp.tile([128, 8], I32, tag="it", name=f"it{p}_{th}")
                    nc.vector.tensor_scalar(
                        it_[:], mm_[:].bitcast(I32), 1, None,
                        OP.logical_shift_right)
                    nc.vector.tensor_scalar(it_[:], it_[:], -1, None,
                                            OP.bitwise_xor)
                    nc.vector.tensor_scalar(it_[:], it_[:], 0x5f3759df + 1,
                                            None, OP.add)
                    y0 = it_[:].bitcast(F32)
                    nw = epp.tile([128, 8], F32, tag="nw", name=f"nw{p}_{th}")
                    nc.vector.tensor_mul(nw[:], y0, y0)
                    nc.vector.tensor_mul(nw[:], nw[:], mm_[:])
                    nc.vector.tensor_scalar(nw[:], nw[:], -0.5, 1.5,
                                            OP.mult, OP.add)
                    rinv = epp.tile([128, 8], F32, tag="rinv",
                                    name=f"rinv{p}_{th}")
                    nc.vector.tensor_mul(rinv[:], y0, nw[:])

                    a16 = epp.tile([128, 512], F16, tag="a16",
                                   name=f"a16_{p}_{th}")
                    rinvB = rinv[:].unsqueeze(2).broadcast_to([128, 8, 64])
                    nc.gpsimd.tensor_mul(
                        a16[:].rearrange("p (t c) -> p t c", c=64),
                        attn[:].rearrange("p (t c) -> p t c", c=64), rinvB)
                    return a16

                state = {}

                def late1_run(state=state, late1=late1):
                    state["a16"] = late1()

                def late2(state=state, row=row, c=p // 2, th=th, p=p):
                    a16 = state["a16"]
                    tp = tpp.tile([128, 1024], F16, tag="tp",
                                  name=f"tp{p}_{th}")
                    for t in range(8):
                        nc.tensor.matmul(
                            tp[row:row + 64, t * 128:(t + 1) * 128],
                            lhsT=a16[:, t * 64:(t + 1) * 64],
                            rhs=ident[:], is_transpose=True,
                            start=(t == 0), stop=(t == 7),
                            skip_group_check=True)
                    nc.scalar.copy(
                        at[c][row:row + 64, th * 1024:(th + 1) * 1024],
                        tp[row:row + 64, :])

                pending = [(4, late1_run), (10, late2)]

        for _, fn in pending:
            fn()
        pending = []

    # ---------------- Phase C: output projection ----------------
    with ExitStack() as cctx:
      if "C" in phases:
        cpool = cctx.enter_context(
            tc.tile_pool(name=f"cpsum{it}", bufs=8, space="PSUM"))
        spool = cctx.enter_context(tc.tile_pool(name=f"cst{it}", bufs=8))
        for e in range(8):
            for nt in range(4):
                ps = cpool.tile([128, 512], F32, tag="op",
                                name=f"ops{e}_{nt}")
                for c in range(2):
                    nc.tensor.matmul(
                        ps[:],
                        lhsT=wo_sb[c][:, e * 128:(e + 1) * 128],
                        rhs=at[c][:, nt * 512:(nt + 1) * 512],
                        start=(c == 0), stop=(c == 1))
                st = spool.tile([128, 512], F16, tag="st",
                                name=f"st{e}_{nt}")
                if (e * 4 + nt) % 2 == 0:
                    nc.scalar.copy(st[:], ps[:])
                else:
                    nc.vector.tensor_copy(st[:], ps[:])
                nc.sync.dma_start(
                    out=out_d[e * 128:(e + 1) * 128,
                              nt * 512:(nt + 1) * 512],
                    in_=st[:])


MERGEZ = True
PRETRANS = True


def _build(lam: float, phases: str = "ABC", unroll: int = 1):
    nc = bacc.Bacc("TRN2", target_bir_lowering=False, debug=False,
                   enable_asserts=False, num_devices=8)

    if PRETRANS:
        xq_d = nc.dram_tensor("xq_t", [EMBED, T], F16, kind="ExternalInput").ap()
        xkv_d = nc.dram_tensor("xkv_t", [EMBED, S], F16, kind="ExternalInput").ap()
    else:
        xq_d = nc.dram_tensor("xq_r", [T, EMBED], F16, kind="ExternalInput").ap()
        xkv_d = nc.dram_tensor("xkv_r", [S, EMBED], F16, kind="ExternalInput").ap()
    wq_d = nc.dram_tensor("wq", [EMBED, HC], F16, kind="ExternalInput").ap()
    wk_d = nc.dram_tensor("wk", [EMBED, HC], F16, kind="ExternalInput").ap()
    wv_d = nc.dram_tensor("wv", [EMBED, HC], F16, kind="ExternalInput").ap()
    wo_d = nc.dram_tensor("wo", [HC, EMBED], F16, kind="ExternalInput").ap()
    out_d = nc.dram_tensor("out_t", [EMBED, T], F16, kind="ExternalOutput").ap()
    dram = (xq_d, xkv_d, wq_d, wk_d, wv_d, wo_d, out_d)

    with tile.TileContext(nc) as tc, ExitStack() as ctx:
        for it in range(unroll):
            with ExitStack() as ictx:
                _emit(nc, tc, ictx, dram, lam, phases, it)

    nc.compile()
    return nc


def _lam_of(inputs):
    lam1 = np.exp(np.sum(np.asarray(inputs["lambda_q1"], np.float32)
                         * np.asarray(inputs["lambda_k1"], np.float32),
                         dtype=np.float32))
    lam2 = np.exp(np.sum(np.asarray(inputs["lambda_q2"], np.float32)
                         * np.asarray(inputs["lambda_k2"], np.float32),
                         dtype=np.float32))
    return float(np.float32(lam1 - lam2 + np.float32(LAMBDA_INIT)))


def _in_maps(inputs):
    q = np.asarray(inputs["query_x"], np.float32)
    kv = np.asarray(inputs["kv_x"], np.float32)
    Wq = np.asarray(inputs["Wq"], np.float32)
    Wk = np.asarray(inputs["Wk"], np.float32)
    Wv = np.asarray(inputs["Wv"], np.float32)
    Wo = np.asarray(inputs["Wo"], np.float32)
    subln_w = np.asarray(inputs["subln_w"], np.float32)

    scaling = np.float32(D ** -0.5)
    wo_scale = (np.tile(subln_w, PAIRS) * np.float32(1.0 - LAMBDA_INIT))
    if PRETRANS:
        xq16 = [np.ascontiguousarray(q[b].T).astype(np.float16)
                for b in range(2)]
        xkv16 = [np.ascontiguousarray(kv[b].T).astype(np.float16)
                 for b in range(2)]
        xq_key, xkv_key = "xq_t", "xkv_t"
    else:
        xq16 = [q[b].astype(np.float16) for b in range(2)]
        xkv16 = [kv[b].astype(np.float16) for b in range(2)]
        xq_key, xkv_key = "xq_r", "xkv_r"
    maps = []
    for core in range(8):
        b, g = divmod(core, 4)
        sl = slice(g * HC, (g + 1) * HC)
        maps.append({
            xq_key: xq16[b],
            xkv_key: xkv16[b],
            "wq": (Wq[:, sl] * scaling).astype(np.float16),
            "wk": Wk[:, sl].astype(np.float16),
            "wv": Wv[:, sl].astype(np.float16),
            "wo": (Wo[sl, :] * wo_scale[:, None]).astype(np.float16),
        })
    return maps


def _get_nc(inputs, phases="ABC", unroll=1):
    lam = _lam_of(inputs)
    key = (round(lam, 12), phases, unroll, MERGEZ, PRETRANS)
    if key not in _cache:
        _cache[key] = _build(lam, phases, unroll)
    return _cache[key]


def _run(inputs):
    nc = _get_nc(inputs)
    in_maps = _in_maps(inputs)
    res = run_bass_kernel_spmd(nc, in_maps, list(range(8)), trace=False)
    out = np.zeros((2, T, EMBED), np.float32)
    for core in range(8):
        out[core // 4] += res.results[core]["out_t"].T.astype(np.float32)
    return out, res


def kernel(**inputs):
    out, _ = _run(inputs)
    return out


def _run_timed(inputs, iters=30, phases="ABC", unroll=32):
    """Times an unroll-N NEFF with device-resident inputs and reports the
    steady-state marginal per-call slope divided by N. The slope between two
    batch sizes cancels the fixed client/axon dispatch overhead; dividing by
    the in-NEFF unroll amortizes the ~0.5ms per-call RPC floor so the number
    approximates true per-kernel device execution time."""
    import time
    import jax
    from jax.experimental.shard_map import shard_map
    from jax.sharding import Mesh, NamedSharding, PartitionSpec
    from concourse import bass2jax, mybir as mb

    nc = _get_nc(inputs, phases, unroll)
    in_maps = _in_maps(inputs)

    bass2jax.install_neuronx_cc_hook()
    n_cores = 8
    partition_name = (nc.partition_id_tensor.name
                      if nc.partition_id_tensor else None)
    in_names, out_names, out_avals, zero_outs = [], [], [], []
    for alloc in nc.m.functions[0].allocations:
        if not isinstance(alloc, mb.MemoryLocationSet):
            continue
        name = alloc.memorylocations[0].name
        if alloc.kind == "ExternalInput":
            if name != partition_name:
                in_names.append(name)
        elif alloc.kind == "ExternalOutput":
            out_names.append(name)
            shape = tuple(alloc.tensor_shape)
            dtype = mb.dt.np(alloc.dtype)
            out_avals.append(jax.core.ShapedArray(shape, dtype))
            zero_outs.append(np.zeros(shape, dtype))
    n_params = len(in_names)
    all_names = in_names + out_names
    if partition_name is not None:
        all_names = all_names + [partition_name]

    def _body(*args):
        operands = list(args)
        if partition_name is not None:
            operands.append(bass2jax.partition_id_tensor())
        outs = bass2jax._bass_exec_p.bind(
            *operands,
            out_avals=tuple(out_avals),
            in_names=tuple(all_names),
            out_names=tuple(out_names),
            lowering_input_output_aliases=(),
            sim_require_finite=True,
            sim_require_nnan=True,
            nc=nc,
        )
        return tuple(outs)

    devices = jax.devices()[:n_cores]
    mesh = Mesh(np.asarray(devices), ("core",))
    spec = NamedSharding(mesh, PartitionSpec("core"))
    n_outs = len(out_names)
    sharded = jax.jit(
        shard_map(_body, mesh=mesh,
                  in_specs=(PartitionSpec("core"),) * (n_params + n_outs),
                  out_specs=(PartitionSpec("core"),) * n_outs,
                  check_rep=False),
        keep_unused=True)

    concat_in = [
        jax.device_put(
            np.concatenate([in_maps[c][nm] for c in range(n_cores)], axis=0),
            spec)
        for nm in in_names
    ]
    concat_zeros = [
        jax.device_put(np.zeros((n_cores * z.shape[0], *z.shape[1:]), z.dtype),
                       spec)
        for z in zero_outs
    ]

    out_arrs = sharded(*concat_in, *concat_zeros)  # compile + first run
    jax.block_until_ready(out_arrs)
    for _ in range(4):  # warmup
        jax.block_until_ready(sharded(*concat_in, *concat_zeros))

    lo, hi = max(10, iters // 3), max(40, iters * 4 // 3)
    best = {}
    for n in (lo, hi):
        b = None
        for _rep in range(4):
            t0 = time.perf_counter()
            rs = [sharded(*concat_in, *concat_zeros) for _ in range(n)]
            jax.block_until_ready(rs)
            t1 = time.perf_counter()
            tt = t1 - t0
            b = tt if b is None else min(b, tt)
        best[n] = b
    marginal_per_call = (best[hi] - best[lo]) / (hi - lo)
    per_kernel_ns = marginal_per_call / unroll * 1e9

    out = np.zeros((2, T, EMBED), np.float32)
    full = np.asarray(out_arrs[0]).reshape(n_cores, EMBED, T)
    for core in range(8):
        out[core // 4] += full[core].T.astype(np.float32)
    return out, per_kernel_ns


# revision 7
# speedup vs baseline: 1.1898x; 1.1898x over previous
"""CrossMultiheadDiffAttn Trainium2 kernel (v3).

Same sharding as v2: 8 cores = 2 batches x 4 head-groups.
New options vs v2:
  pretrans: host pre-transposes x inputs to [E, T]; straight DMA loads
            (no XBAR dma_start_transpose).
  mergez:   softmax denominator column merged into the AV matmul rhs
            ([vv | ones] 65-wide slots in 3 PSUM bank tiles) instead of
            2048 separate 1-column matmuls (halves PE weight loads).
  unroll:   emit the whole body N times in one NEFF (for timing: amortizes
            the per-call RPC floor so device exec becomes measurable).
"""

import math

import numpy as np

import concourse.bacc as bacc
import concourse.bass as bass
import concourse.mybir as mybir
import concourse.tile as tile
from concourse import masks
from concourse.bass_utils import run_bass_kernel_spmd
from contextlib import ExitStack

F32 = mybir.dt.float32
F16 = mybir.dt.float16
I16 = mybir.dt.int16
I32 = mybir.dt.int32
AF = mybir.ActivationFunctionType
OP = mybir.AluOpType

EMBED = 1024
HEADS = 16
DEPTH = 12
D = 32
LAMBDA_INIT = 0.8 - 0.6 * math.exp(-0.3 * DEPTH)
T = 2048
S = 2048
HC = 256
NE = EMBED // 128
NS = S // 128
PAIRS = 4
EPS = 1e-5

ACOL = 512
SCH_A = 1024.0 / math.log(2.0)
SCH_B = 15360.0 - 47.0

_cache = {}


def _emit(nc, tc, ctx, dram, lam, phases, it):
    """Emit one full iteration of the kernel body. `it` suffixes pool names."""
    xq_d, xkv_d, wq_d, wk_d, wv_d, wo_d, out_d = dram
    pretrans = xq_d.shape[0] == EMBED

    persist = ctx.enter_context(tc.tile_pool(name=f"persist{it}", bufs=1))
    ident = persist.tile([128, 128], F16, tag="ident", name="ident")
    masks.make_identity(nc, ident[:])
    ones16 = persist.tile([128, 1], F16, tag="ones", name="ones")
    nc.vector.memset(ones16[:], 1.0)
    nlam16 = persist.tile([128, 1], F16, tag="nlam", name="nlam")
    nc.vector.memset(nlam16[:], -1.0 / lam)

    qt = [persist.tile([128, T], F16, tag=f"qt{m}", name=f"qt{m}")
          for m in range(2)]
    kt = [persist.tile([128, S], F16, tag=f"kt{m}", name=f"kt{m}")
          for m in range(2)]
    # vv tiles: [vv(64 per pair) | ones] 65-wide groups when mergez
    vw = 65 if MERGEZ else 64
    vv = [persist.tile([128, 4 * vw], F16, tag=f"vv{s}", name=f"vv{s}")
          for s in range(NS)]
    at = [persist.tile([128, T], F16, tag=f"at{c}", name=f"at{c}")
          for c in range(2)]
    wo_sb = [persist.tile([128, EMBED], F16, tag=f"wo{c}", name=f"wo{c}")
             for c in range(2)]

    # ---------------- Phase A: loads + projections ----------------
    with ExitStack() as actx:
        apool = actx.enter_context(tc.tile_pool(name=f"ain{it}", bufs=1))
        xkv = [apool.tile([128, S], F16, tag=f"xkv{e}", name=f"xkv{e}")
               for e in range(NE)]
        xq = [apool.tile([128, T], F16, tag=f"xq{e}", name=f"xq{e}")
              for e in range(NE)]
        wk_sb = [apool.tile([128, HC], F16, tag=f"wk{e}", name=f"wk{e}")
                 for e in range(NE)]
        wv_sb = [apool.tile([128, HC], F16, tag=f"wv{e}", name=f"wv{e}")
                 for e in range(NE)]
        wq_sb = [apool.tile([128, HC], F16, tag=f"wq{e}", name=f"wq{e}")
                 for e in range(NE)]

        if pretrans:
            nc.sync.dma_start(out=xkv[0][:], in_=xkv_d[0:128, :])
        for e in range(NE):
            nc.sync.dma_start(out=wv_sb[e][:], in_=wv_d[e * 128:(e + 1) * 128, :])
        for e in range(NE):
            if pretrans:
                if e > 0:
                    nc.sync.dma_start(out=xkv[e][:],
                                      in_=xkv_d[e * 128:(e + 1) * 128, :])
            else:
                nc.sync.dma_start_transpose(
                    out=xkv[e][:], in_=xkv_d[:, e * 128:(e + 1) * 128])
        for e in range(NE):
            nc.sync.dma_start(out=wk_sb[e][:], in_=wk_d[e * 128:(e + 1) * 128, :])
            nc.sync.dma_start(out=wq_sb[e][:], in_=wq_d[e * 128:(e + 1) * 128, :])
        for e in range(NE):
            if pretrans:
                nc.sync.dma_start(out=xq[e][:],
                                  in_=xq_d[e * 128:(e + 1) * 128, :])
            else:
                nc.sync.dma_start_transpose(
                    out=xq[e][:], in_=xq_d[:, e * 128:(e + 1) * 128])
        for c in range(2):
            nc.sync.dma_start(out=wo_sb[c][:], in_=wo_d[c * 128:(c + 1) * 128, :])

        with tc.tile_pool(name=f"vpsum{it}", bufs=1, space="PSUM") as vpool:
            vps = [vpool.tile([128, 512], F32, tag=f"vps{j}",
                              name=f"vps{j}") for j in range(8)]
            for e in range(NE):
                for s in range(NS):
                    j, half = divmod(s, 2)
                    nc.tensor.matmul(
                        vps[j][:, half * HC:(half + 1) * HC],
                        lhsT=xkv[e][:, s * 128:(s + 1) * 128],
                        rhs=wv_sb[e][:],
                        start=(e == 0 and half == 0),
                        stop=(e == NE - 1 and half == 1),
                        skip_group_check=True)
            for s in range(NS):
                j, half = divmod(s, 2)
                if MERGEZ:
                    for p in range(PAIRS):
                        nc.scalar.copy(
                            vv[s][:, p * 65:p * 65 + 64],
                            vps[j][:, half * HC + p * 64:half * HC + (p + 1) * 64])
                    nc.vector.memset(vv[s][:, 64:4 * 65:65], 1.0)
                else:
                    nc.scalar.copy(vv[s][:], vps[j][:, half * HC:(half + 1) * HC])

        ppool = actx.enter_context(
            tc.tile_pool(name=f"ppsum{it}", bufs=4, space="PSUM"))

        for w_sb, x_sb, dst in ((wk_sb, xkv, kt), (wq_sb, xq, qt)):
            for m in range(2):
                for n in range(4):
                    ps = ppool.tile([128, 512], F32, tag="pj",
                                    name=f"qkps{m}_{n}")
                    for e in range(NE):
                        nc.tensor.matmul(
                            ps[:],
                            lhsT=w_sb[e][:, m * 128:(m + 1) * 128],
                            rhs=x_sb[e][:, n * 512:(n + 1) * 512],
                            start=(e == 0), stop=(e == NE - 1))
                    nc.vector.tensor_copy(dst[m][:, n * 512:(n + 1) * 512],
                                          ps[:])

    if "B" not in phases:
        for m in range(2):
            st = persist.tile([128, T], F16, tag=f"dbg{m}", name=f"dbg{m}")
            nc.vector.tensor_copy(st[:], qt[m][:])
            nc.sync.dma_start(out=out_d[m * 128:(m + 1) * 128, :], in_=st[:])
            st2 = persist.tile([128, T], F16, tag=f"dbg2{m}", name=f"dbg2{m}")
            nc.vector.tensor_copy(st2[:], kt[m][:])
            nc.sync.dma_start(out=out_d[(2 + m) * 128:(3 + m) * 128, :],
                              in_=st2[:])

    # ---------------- Phase B: attention ----------------
    with ExitStack() as bctx:
      if "B" in phases:
        scp = bctx.enter_context(
            tc.tile_pool(name=f"scp{it}", bufs=2, space="PSUM"))
        avp = bctx.enter_context(
            tc.tile_pool(name=f"avp{it}", bufs=1, space="PSUM"))
        if not MERGEZ:
            ztp = bctx.enter_context(
                tc.tile_pool(name=f"ztp{it}", bufs=1, space="PSUM"))
        tpp = bctx.enter_context(
            tc.tile_pool(name=f"tpp{it}", bufs=1, space="PSUM"))
        ptp = bctx.enter_context(tc.tile_pool(name=f"ptp{it}", bufs=5))
        epp = bctx.enter_context(tc.tile_pool(name=f"epp{it}", bufs=3))

        pending = []

        for p in range(PAIRS):
            m = p // 2
            row = (p % 2) * 64
            for th in range(2):
                if MERGEZ:
                    # 16 slots of 65 (8 t-chunks x 2 hi), 6 per bank tile
                    avb = [avp.tile([128, 512], F32, tag=f"avb{j}",
                                    name=f"avb{j}_{p}_{th}") for j in range(3)]

                    def av_slot(hi, t, avb=avb):
                        k = hi * 8 + t
                        j, kk = divmod(k, 6)
                        return avb[j][:, kk * 65:(kk + 1) * 65]
                else:
                    av = avp.tile([128, 1024], F32, tag="av",
                                  name=f"av{p}_{th}")
                    zt = ztp.tile([128, 16], F32, tag="zt", name=f"zt{p}_{th}")
                units = [(s, hi) for s in range(NS) for hi in range(2)]
                sc_t, pt_t = {}, {}

                def emit_sc_exp(u, p=p, th=th, m=m, sc_t=sc_t, pt_t=pt_t):
                    s, hi = u
                    h = 2 * p + hi
                    qo = (h % 4) * 32
                    sca = scp.tile([128, 512], F32, tag="sca",
                                   name=f"sca{p}_{th}_{s}_{hi}")
                    scd = scp.tile([128, 512], F32, tag="scd",
                                   name=f"scd{p}_{th}_{s}_{hi}")
                    for nj, sc in ((0, sca), (1, scd)):
                        nc.tensor.matmul(
                            sc[:],
                            lhsT=kt[m][qo:qo + 32, s * 128:(s + 1) * 128],
                            rhs=qt[m][qo:qo + 32,
                                      th * 1024 + nj * 512:
                                      th * 1024 + (nj + 1) * 512],
                            start=True, stop=True, tile_position=(qo, 0))
                    pta = ptp.tile([128, ACOL], F16, tag="pta",
                                   name=f"pta{p}_{th}_{s}_{hi}")
                    ptd = ptp.tile([128, 1024 - ACOL], I16, tag="ptd",
                                   name=f"ptd{p}_{th}_{s}_{hi}")
                    nc.scalar.activation(pta[:], sca[:], AF.Exp)
                    nc.vector.tensor_scalar(
                        ptd[:], scd[:],
                        SCH_A, SCH_B, OP.mult, OP.add)
                    sc_t[u] = (sca, scd)
                    pt_t[u] = (pta, ptd)

                nsplit = ACOL // 128

                def pt_chunk(pt, t):
                    pta, ptd = pt
                    if t < nsplit:
                        return pta[:, t * 128:(t + 1) * 128]
                    return ptd[:, (t - nsplit) * 128:
                               (t - nsplit + 1) * 128].bitcast(F16)

                if MERGEZ:
                    def emit_av(u, p=p, th=th, sc_t=sc_t, pt_t=pt_t,
                                av_slot=av_slot):
                        s, hi = u
                        pt = pt_t.pop(u)
                        del sc_t[u]
                        for t in range(8):
                            k = hi * 8 + t
                            nc.tensor.matmul(
                                av_slot(hi, t),
                                lhsT=pt_chunk(pt, t),
                                rhs=vv[s][:, p * 65:(p + 1) * 65],
                                start=(s == 0 and k in (0, 6, 12)),
                                stop=(s == NS - 1 and k in (5, 11, 15)),
                                skip_group_check=True)
                else:
                    def emit_av(u, p=p, av=av, zt=zt, sc_t=sc_t, pt_t=pt_t):
                        s, hi = u
                        pt = pt_t.pop(u)
                        del sc_t[u]
                        for t in range(8):
                            nc.tensor.matmul(
                                av[:, hi * 512 + t * 64:hi * 512 + (t + 1) * 64],
                                lhsT=pt_chunk(pt, t),
                                rhs=vv[s][:, p * 64:(p + 1) * 64],
                                start=(s == 0 and t == 0),
                                stop=(s == NS - 1 and t == 7),
                                skip_group_check=True)
                        for t in range(8):
                            nc.tensor.matmul(
                                zt[:, hi * 8 + t:hi * 8 + t + 1],
                                lhsT=pt_chunk(pt, t),
                                rhs=ones16[:],
                                start=(s == 0 and hi == 0 and t == 0),
                                stop=(s == NS - 1 and hi == 1 and t == 7),
                                skip_group_check=True)

                emit_sc_exp(units[0])
                for i, u in enumerate(units):
                    if i + 1 < len(units):
                        emit_sc_exp(units[i + 1])
                    emit_av(u)
                    while pending and pending[0][0] <= i:
                        pending.pop(0)[1]()

                # ---- epilogue ----
                rec = epp.tile([128, 16], F32, tag="rec",
                               name=f"rec{p}_{th}")
                if MERGEZ:
                    # gather z cols (slot col 64, stride 65) from 3 bank tiles
                    nc.vector.reciprocal(
                        rec[:, 0:6], avb[0][:, 64:64 + 6 * 65:65])
                    nc.vector.reciprocal(
                        rec[:, 6:12], avb[1][:, 64:64 + 6 * 65:65])
                    nc.vector.reciprocal(
                        rec[:, 12:16], avb[2][:, 64:64 + 4 * 65:65])
                else:
                    nc.vector.reciprocal(rec[:], zt[:, 0:16])
                rec2l = epp.tile([128, 8], F32, tag="rec2l",
                                 name=f"rec2l{p}_{th}")
                nc.vector.tensor_scalar_mul(rec2l[:], rec[:, 8:16], -lam)

                tmp = epp.tile([128, 512], F32, tag="tmp",
                               name=f"tmp{p}_{th}")
                e2 = epp.tile([128, 512], F32, tag="e2", name=f"e2{p}_{th}")
                if MERGEZ:
                    # slots k=0..15 (65-wide, 6|6|4 per bank tile); o-part is
                    # a 3D strided view, scaled by the per-partition rec col.
                    def oview(j, a, n):
                        return avb[j][:, a * 65:(a + n) * 65].rearrange(
                            "p (t c) -> p t c", c=65)[:, :, 0:64]
                    def rview(r, a, n):
                        return r[:, a:a + n].unsqueeze(2).broadcast_to(
                            [128, n, 64])
                    t3 = tmp[:].rearrange("p (t c) -> p t c", c=64)
                    e3 = e2[:].rearrange("p (t c) -> p t c", c=64)
                    nc.vector.tensor_mul(t3[:, 0:6], oview(0, 0, 6),
                                         rview(rec, 0, 6))
                    nc.vector.tensor_mul(t3[:, 6:8], oview(1, 0, 2),
                                         rview(rec, 6, 2))
                    nc.vector.tensor_mul(e3[:, 0:4], oview(1, 2, 4),
                                         rview(rec2l, 0, 4))
                    nc.vector.tensor_mul(e3[:, 4:8], oview(2, 0, 4),
                                         rview(rec2l, 4, 4))
                else:
                    o1 = av[:, 0:512].rearrange("p (t c) -> p t c", c=64)
                    o2 = av[:, 512:1024].rearrange("p (t c) -> p t c", c=64)
                    recB = rec[:, 0:8].unsqueeze(2).broadcast_to([128, 8, 64])
                    rec2B = rec2l[:].unsqueeze(2).broadcast_to([128, 8, 64])
                    nc.vector.tensor_mul(
                        tmp[:].rearrange("p (t c) -> p t c", c=64), o1, recB)
                    nc.vector.tensor_mul(
                        e2[:].rearrange("p (t c) -> p t c", c=64), o2, rec2B)

                def late1(tmp=tmp, e2=e2, p=p, th=th):
                    attn = epp.tile([128, 512], F32, tag="attn",
                                    name=f"attn{p}_{th}")
                    nc.gpsimd.tensor_add(attn[:], tmp[:], e2[:])
                    sq = epp.tile([128, 512], F32, tag="sq",
                                  name=f"sq{p}_{th}")
                    nc.gpsimd.tensor_mul(sq[:], attn[:], attn[:])
                    ssq = epp.tile([128, 8], F32, tag="ssq",
                                   name=f"ssq{p}_{th}")
                    nc.vector.reduce_sum(
                        ssq[:], sq[:].rearrange("p (t c) -> p t c", c=64),
                        axis=mybir.AxisListType.X)
                    mm_ = epp.tile([128, 8], F32, tag="mm", name=f"mm{p}_{th}")
                    nc.vector.tensor_scalar(mm_[:], ssq[:], 1.0 / 64.0,
                                            EPS, OP.mult, OP.add)
                    it_ = epp.tile([128, 8], I32, tag="it", name=f"it{p}_{th}")
                    nc.vector.tensor_scalar(
                        it_[:], mm_[:].bitcast(I32), 1, None,
                        OP.logical_shift_right)
                    nc.vector.tensor_scalar(it_[:], it_[:], -1, None,
                                            OP.bitwise_xor)
                    nc.vector.tensor_scalar(it_[:], it_[:], 0x5f3759df + 1,
                                            None, OP.add)
                    y0 = it_[:].bitcast(F32)
                    nw = epp.tile([128, 8], F32, tag="nw", name=f"nw{p}_{th}")
                    nc.vector.tensor_mul(nw[:], y0, y0)
                    nc.vector.tensor_mul(nw[:], nw[:], mm_[:])
                    nc.vector.tensor_scalar(nw[:], nw[:], -0.5, 1.5,
                                            OP.mult, OP.add)
                    rinv = epp.tile([128, 8], F32, tag="rinv",
                                    name=f"rinv{p}_{th}")
                    nc.vector.tensor_mul(rinv[:], y0, nw[:])

                    a16 = epp.tile([128, 512], F16, tag="a16",
                                   name=f"a16_{p}_{th}")
                    rinvB = rinv[:].unsqueeze(2).broadcast_to([128, 8, 64])
                    nc.gpsimd.tensor_mul(
                        a16[:].rearrange("p (t c) -> p t c", c=64),
                        attn[:].rearrange("p (t c) -> p t c", c=64), rinvB)
                    return a16

                state = {}

                def late1_run(state=state, late1=late1):
                    state["a16"] = late1()

                def late2(state=state, row=row, c=p // 2, th=th, p=p):
                    a16 = state["a16"]
                    tp = tpp.tile([128, 1024], F16, tag="tp",
                                  name=f"tp{p}_{th}")
                    for t in range(8):
                        nc.tensor.matmul(
                            tp[row:row + 64, t * 128:(t + 1) * 128],
                            lhsT=a16[:, t * 64:(t + 1) * 64],
                            rhs=ident[:], is_transpose=True,
                            start=(t == 0), stop=(t == 7),
                            skip_group_check=True)
                    nc.scalar.copy(
                        at[c][row:row + 64, th * 1024:(th + 1) * 1024],
                        tp[row:row + 64, :])

                pending = [(4, late1_run), (10, late2)]

        for _, fn in pending:
            fn()
        pending = []

    # ---------------- Phase C: output projection ----------------
    with ExitStack() as cctx:
      if "C" in phases:
        cpool = cctx.enter_context(
            tc.tile_pool(name=f"cpsum{it}", bufs=8, space="PSUM"))
        spool = cctx.enter_context(tc.tile_pool(name=f"cst{it}", bufs=8))
        for e in range(8):
            for nt in range(4):
                ps = cpool.tile([128, 512], F32, tag="op",
                                name=f"ops{e}_{nt}")
                for c in range(2):
                    nc.tensor.matmul(
                        ps[:],
                        lhsT=wo_sb[c][:, e * 128:(e + 1) * 128],
                        rhs=at[c][:, nt * 512:(nt + 1) * 512],
                        start=(c == 0), stop=(c == 1))
                st = spool.tile([128, 512], F16, tag="st",
                                name=f"st{e}_{nt}")
                if (e * 4 + nt) % 2 == 0:
                    nc.scalar.copy(st[:], ps[:])
                else:
                    nc.vector.tensor_copy(st[:], ps[:])
                nc.sync.dma_start(
                    out=out_d[e * 128:(e + 1) * 128,
                              nt * 512:(nt + 1) * 512],
                    in_=st[:])


MERGEZ = True
PRETRANS = True


def _build(lam: float, phases: str = "ABC", unroll: int = 1):
    nc = bacc.Bacc("TRN2", target_bir_lowering=False, debug=False,
                   enable_asserts=False, num_devices=8)

    if PRETRANS:
        xq_d = nc.dram_tensor("xq_t", [EMBED, T], F16, kind="ExternalInput").ap()
        xkv_d = nc.dram_tensor("xkv_t", [EMBED, S], F16, kind="ExternalInput").ap()
    else:
        xq_d = nc.dram_tensor("xq_r", [T, EMBED], F16, kind="ExternalInput").ap()
        xkv_d = nc.dram_tensor("xkv_r", [S, EMBED], F16, kind="ExternalInput").ap()
    wq_d = nc.dram_tensor("wq", [EMBED, HC], F16, kind="ExternalInput").ap()
    wk_d = nc.dram_tensor("wk", [EMBED, HC], F16, kind="ExternalInput").ap()
    wv_d = nc.dram_tensor("wv", [EMBED, HC], F16, kind="ExternalInput").ap()
    wo_d = nc.dram_tensor("wo", [HC, EMBED], F16, kind="ExternalInput").ap()
    out_d = nc.dram_tensor("out_t", [EMBED, T], F16, kind="ExternalOutput").ap()
    dram = (xq_d, xkv_d, wq_d, wk_d, wv_d, wo_d, out_d)

    with tile.TileContext(nc) as tc, ExitStack() as ctx:
        for it in range(unroll):
            with ExitStack() as ictx:
                _emit(nc, tc, ictx, dram, lam, phases, it)

    nc.compile()
    return nc


def _lam_of(inputs):
    lam1 = np.exp(np.sum(np.asarray(inputs["lambda_q1"], np.float32)
                         * np.asarray(inputs["lambda_k1"], np.float32),
                         dtype=np.float32))
    lam2 = np.exp(np.sum(np.asarray(inputs["lambda_q2"], np.float32)
                         * np.asarray(inputs["lambda_k2"], np.float32),
                         dtype=np.float32))
    return float(np.float32(lam1 - lam2 + np.float32(LAMBDA_INIT)))


def _in_maps(inputs):
    q = np.asarray(inputs["query_x"], np.float32)
    kv = np.asarray(inputs["kv_x"], np.float32)
    Wq = np.asarray(inputs["Wq"], np.float32)
    Wk = np.asarray(inputs["Wk"], np.float32)
    Wv = np.asarray(inputs["Wv"], np.float32)
    Wo = np.asarray(inputs["Wo"], np.float32)
    subln_w = np.asarray(inputs["subln_w"], np.float32)

    scaling = np.float32(D ** -0.5)
    wo_scale = (np.tile(subln_w, PAIRS) * np.float32(1.0 - LAMBDA_INIT))
    if PRETRANS:
        xq16 = [np.ascontiguousarray(q[b].T).astype(np.float16)
                for b in range(2)]
        xkv16 = [np.ascontiguousarray(kv[b].T).astype(np.float16)
                 for b in range(2)]
        xq_key, xkv_key = "xq_t", "xkv_t"
    else:
        xq16 = [q[b].astype(np.float16) for b in range(2)]
        xkv16 = [kv[b].astype(np.float16) for b in range(2)]
        xq_key, xkv_key = "xq_r", "xkv_r"
    maps = []
    for core in range(8):
        b, g = divmod(core, 4)
        sl = slice(g * HC, (g + 1) * HC)
        maps.append({
            xq_key: xq16[b],
            xkv_key: xkv16[b],
            "wq": (Wq[:, sl] * scaling).astype(np.float16),
            "wk": Wk[:, sl].astype(np.float16),
            "wv": Wv[:, sl].astype(np.float16),
            "wo": (Wo[sl, :] * wo_scale[:, None]).astype(np.float16),
        })
    return maps


def _get_nc(inputs, phases="ABC", unroll=1):
    lam = _lam_of(inputs)
    key = (round(lam, 12), phases, unroll, MERGEZ, PRETRANS)
    if key not in _cache:
        _cache[key] = _build(lam, phases, unroll)
    return _cache[key]


def _run(inputs):
    nc = _get_nc(inputs)
    in_maps = _in_maps(inputs)
    res = run_bass_kernel_spmd(nc, in_maps, list(range(8)), trace=False)
    out = np.zeros((2, T, EMBED), np.float32)
    for core in range(8):
        out[core // 4] += res.results[core]["out_t"].T.astype(np.float32)
    return out, res


def kernel(**inputs):
    out, _ = _run(inputs)
    return out


def _run_timed(inputs, iters=30, phases="ABC", unroll=32):
    """Times an unroll-N NEFF with device-resident inputs and reports the
    steady-state marginal per-call slope divided by N. The slope between two
    batch sizes cancels the fixed client/axon dispatch overhead; dividing by
    the in-NEFF unroll amortizes the ~0.5ms per-call RPC floor so the number
    approximates true per-kernel device execution time."""
    import time
    import jax
    from jax.experimental.shard_map import shard_map
    from jax.sharding import Mesh, NamedSharding, PartitionSpec
    from concourse import bass2jax, mybir as mb

    nc = _get_nc(inputs, phases, unroll)
    in_maps = _in_maps(inputs)

    bass2jax.install_neuronx_cc_hook()
    n_cores = 8
    partition_name = (nc.partition_id_tensor.name
                      if nc.partition_id_tensor else None)
    in_names, out_names, out_avals, zero_outs = [], [], [], []
    for alloc in nc.m.functions[0].allocations:
        if not isinstance(alloc, mb.MemoryLocationSet):
            continue
        name = alloc.memorylocations[0].name
        if alloc.kind == "ExternalInput":
            if name != partition_name:
                in_names.append(name)
        elif alloc.kind == "ExternalOutput":
            out_names.append(name)
            shape = tuple(alloc.tensor_shape)
            dtype = mb.dt.np(alloc.dtype)
            out_avals.append(jax.core.ShapedArray(shape, dtype))
            zero_outs.append(np.zeros(shape, dtype))
    n_params = len(in_names)
    all_names = in_names + out_names
    if partition_name is not None:
        all_names = all_names + [partition_name]

    def _body(*args):
        operands = list(args)
        if partition_name is not None:
            operands.append(bass2jax.partition_id_tensor())
        outs = bass2jax._bass_exec_p.bind(
            *operands,
            out_avals=tuple(out_avals),
            in_names=tuple(all_names),
            out_names=tuple(out_names),
            lowering_input_output_aliases=(),
            sim_require_finite=True,
            sim_require_nnan=True,
            nc=nc,
        )
        return tuple(outs)

    devices = jax.devices()[:n_cores]
    mesh = Mesh(np.asarray(devices), ("core",))
    spec = NamedSharding(mesh, PartitionSpec("core"))
    n_outs = len(out_names)
    sharded = jax.jit(
        shard_map(_body, mesh=mesh,
                  in_specs=(PartitionSpec("core"),) * (n_params + n_outs),
                  out_specs=(PartitionSpec("core"),) * n_outs,
                  check_rep=False),
        keep_unused=True)

    concat_in = [
        jax.device_put(
            np.concatenate([in_maps[c][nm] for c in range(n_cores)], axis=0),
            spec)
        for nm in in_names
    ]
    concat_zeros = [
        jax.device_put(np.zeros((n_cores * z.shape[0], *z.shape[1:]), z.dtype),
                       spec)
        for z in zero_outs
    ]

    out_arrs = sharded(*concat_in, *concat_zeros)  # compile + first run
    jax.block_until_ready(out_arrs)
    for _ in range(4):  # warmup
        jax.block_until_ready(sharded(*concat_in, *concat_zeros))

    lo, hi = max(20, iters // 3), max(60, iters * 2)
    best = {}
    for n in (lo, hi):
        b = None
        for _rep in range(6):
            t0 = time.perf_counter()
            rs = [sharded(*concat_in, *concat_zeros) for _ in range(n)]
            jax.block_until_ready(rs)
            t1 = time.perf_counter()
            tt = t1 - t0
            b = tt if b is None else min(b, tt)
        best[n] = b
    marginal_per_call = (best[hi] - best[lo]) / (hi - lo)
    per_kernel_ns = marginal_per_call / unroll * 1e9

    out = np.zeros((2, T, EMBED), np.float32)
    full = np.asarray(out_arrs[0]).reshape(n_cores, EMBED, T)
    for core in range(8):
        out[core // 4] += full[core].T.astype(np.float32)
    return out, per_kernel_ns


# revision 8
# speedup vs baseline: 1.1951x; 1.0044x over previous
"""CrossMultiheadDiffAttn Trainium2 kernel (v3).

Same sharding as v2: 8 cores = 2 batches x 4 head-groups.
New options vs v2:
  pretrans: host pre-transposes x inputs to [E, T]; straight DMA loads
            (no XBAR dma_start_transpose).
  mergez:   softmax denominator column merged into the AV matmul rhs
            ([vv | ones] 65-wide slots in 3 PSUM bank tiles) instead of
            2048 separate 1-column matmuls (halves PE weight loads).
  unroll:   emit the whole body N times in one NEFF (for timing: amortizes
            the per-call RPC floor so device exec becomes measurable).
"""

import math

import numpy as np

import concourse.bacc as bacc
import concourse.bass as bass
import concourse.mybir as mybir
import concourse.tile as tile
from concourse import masks
from concourse.bass_utils import run_bass_kernel_spmd
from contextlib import ExitStack

F32 = mybir.dt.float32
F16 = mybir.dt.float16
I16 = mybir.dt.int16
I32 = mybir.dt.int32
AF = mybir.ActivationFunctionType
OP = mybir.AluOpType

EMBED = 1024
HEADS = 16
DEPTH = 12
D = 32
LAMBDA_INIT = 0.8 - 0.6 * math.exp(-0.3 * DEPTH)
T = 2048
S = 2048
HC = 256
NE = EMBED // 128
NS = S // 128
PAIRS = 4
EPS = 1e-5

ACOL = 512
SCH_A = 1024.0 / math.log(2.0)
SCH_B = 15360.0 - 47.0

_cache = {}


def _emit(nc, tc, ctx, dram, lam, phases, it):
    """Emit one full iteration of the kernel body. `it` suffixes pool names."""
    xq_d, xkv_d, wq_d, wk_d, wv_d, wo_d, out_d = dram
    pretrans = xq_d.shape[0] == EMBED

    persist = ctx.enter_context(tc.tile_pool(name=f"persist{it}", bufs=1))
    ident = persist.tile([128, 128], F16, tag="ident", name="ident")
    masks.make_identity(nc, ident[:])
    ones16 = persist.tile([128, 1], F16, tag="ones", name="ones")
    nc.vector.memset(ones16[:], 1.0)
    nlam16 = persist.tile([128, 1], F16, tag="nlam", name="nlam")
    nc.vector.memset(nlam16[:], -1.0 / lam)

    qt = [persist.tile([128, T], F16, tag=f"qt{m}", name=f"qt{m}")
          for m in range(2)]
    kt = [persist.tile([128, S], F16, tag=f"kt{m}", name=f"kt{m}")
          for m in range(2)]
    # vv tiles: [vv(64 per pair) | ones] 65-wide groups when mergez
    vw = 65 if MERGEZ else 64
    vv = [persist.tile([128, 4 * vw], F16, tag=f"vv{s}", name=f"vv{s}")
          for s in range(NS)]
    at = [persist.tile([128, T], F16, tag=f"at{c}", name=f"at{c}")
          for c in range(2)]
    wo_sb = [persist.tile([128, EMBED], F16, tag=f"wo{c}", name=f"wo{c}")
             for c in range(2)]

    # ---------------- Phase A: loads + projections ----------------
    with ExitStack() as actx:
        apool = actx.enter_context(tc.tile_pool(name=f"ain{it}", bufs=1))
        xkv = [apool.tile([128, S], F16, tag=f"xkv{e}", name=f"xkv{e}")
               for e in range(NE)]
        xq = [apool.tile([128, T], F16, tag=f"xq{e}", name=f"xq{e}")
              for e in range(NE)]
        wk_sb = [apool.tile([128, HC], F16, tag=f"wk{e}", name=f"wk{e}")
                 for e in range(NE)]
        wv_sb = [apool.tile([128, HC], F16, tag=f"wv{e}", name=f"wv{e}")
                 for e in range(NE)]
        wq_sb = [apool.tile([128, HC], F16, tag=f"wq{e}", name=f"wq{e}")
                 for e in range(NE)]

        if pretrans:
            nc.sync.dma_start(out=xkv[0][:], in_=xkv_d[0:128, :])
        for e in range(NE):
            nc.sync.dma_start(out=wv_sb[e][:], in_=wv_d[e * 128:(e + 1) * 128, :])
        for e in range(NE):
            if pretrans:
                if e > 0:
                    nc.sync.dma_start(out=xkv[e][:],
                                      in_=xkv_d[e * 128:(e + 1) * 128, :])
            else:
                nc.sync.dma_start_transpose(
                    out=xkv[e][:], in_=xkv_d[:, e * 128:(e + 1) * 128])
        for e in range(NE):
            nc.sync.dma_start(out=wk_sb[e][:], in_=wk_d[e * 128:(e + 1) * 128, :])
            nc.sync.dma_start(out=wq_sb[e][:], in_=wq_d[e * 128:(e + 1) * 128, :])
        for e in range(NE):
            if pretrans:
                nc.sync.dma_start(out=xq[e][:],
                                  in_=xq_d[e * 128:(e + 1) * 128, :])
            else:
                nc.sync.dma_start_transpose(
                    out=xq[e][:], in_=xq_d[:, e * 128:(e + 1) * 128])
        for c in range(2):
            nc.sync.dma_start(out=wo_sb[c][:], in_=wo_d[c * 128:(c + 1) * 128, :])

        with tc.tile_pool(name=f"vpsum{it}", bufs=1, space="PSUM") as vpool:
            vps = [vpool.tile([128, 512], F32, tag=f"vps{j}",
                              name=f"vps{j}") for j in range(8)]
            for e in range(NE):
                for s in range(NS):
                    j, half = divmod(s, 2)
                    nc.tensor.matmul(
                        vps[j][:, half * HC:(half + 1) * HC],
                        lhsT=xkv[e][:, s * 128:(s + 1) * 128],
                        rhs=wv_sb[e][:],
                        start=(e == 0 and half == 0),
                        stop=(e == NE - 1 and half == 1),
                        skip_group_check=True)
            for s in range(NS):
                j, half = divmod(s, 2)
                if MERGEZ:
                    for p in range(PAIRS):
                        nc.scalar.copy(
                            vv[s][:, p * 65:p * 65 + 64],
                            vps[j][:, half * HC + p * 64:half * HC + (p + 1) * 64])
                    nc.vector.memset(vv[s][:, 64:4 * 65:65], 1.0)
                else:
                    nc.scalar.copy(vv[s][:], vps[j][:, half * HC:(half + 1) * HC])

        ppool = actx.enter_context(
            tc.tile_pool(name=f"ppsum{it}", bufs=4, space="PSUM"))

        for w_sb, x_sb, dst in ((wk_sb, xkv, kt), (wq_sb, xq, qt)):
            for m in range(2):
                for n in range(4):
                    ps = ppool.tile([128, 512], F32, tag="pj",
                                    name=f"qkps{m}_{n}")
                    for e in range(NE):
                        nc.tensor.matmul(
                            ps[:],
                            lhsT=w_sb[e][:, m * 128:(m + 1) * 128],
                            rhs=x_sb[e][:, n * 512:(n + 1) * 512],
                            start=(e == 0), stop=(e == NE - 1))
                    nc.vector.tensor_copy(dst[m][:, n * 512:(n + 1) * 512],
                                          ps[:])

    if "B" not in phases:
        for m in range(2):
            st = persist.tile([128, T], F16, tag=f"dbg{m}", name=f"dbg{m}")
            nc.vector.tensor_copy(st[:], qt[m][:])
            nc.sync.dma_start(out=out_d[m * 128:(m + 1) * 128, :], in_=st[:])
            st2 = persist.tile([128, T], F16, tag=f"dbg2{m}", name=f"dbg2{m}")
            nc.vector.tensor_copy(st2[:], kt[m][:])
            nc.sync.dma_start(out=out_d[(2 + m) * 128:(3 + m) * 128, :],
                              in_=st2[:])

    # ---------------- Phase B: attention ----------------
    with ExitStack() as bctx:
      if "B" in phases:
        scp = bctx.enter_context(
            tc.tile_pool(name=f"scp{it}", bufs=2, space="PSUM"))
        avp = bctx.enter_context(
            tc.tile_pool(name=f"avp{it}", bufs=1, space="PSUM"))
        if not MERGEZ:
            ztp = bctx.enter_context(
                tc.tile_pool(name=f"ztp{it}", bufs=1, space="PSUM"))
        tpp = bctx.enter_context(
            tc.tile_pool(name=f"tpp{it}", bufs=1, space="PSUM"))
        ptp = bctx.enter_context(tc.tile_pool(name=f"ptp{it}", bufs=5))
        epp = bctx.enter_context(tc.tile_pool(name=f"epp{it}", bufs=3))

        pending = []

        for p in range(PAIRS):
            m = p // 2
            row = (p % 2) * 64
            for th in range(2):
                if MERGEZ:
                    # 16 slots of 65 (8 t-chunks x 2 hi), 6 per bank tile
                    avb = [avp.tile([128, 512], F32, tag=f"avb{j}",
                                    name=f"avb{j}_{p}_{th}") for j in range(3)]

                    def av_slot(hi, t, avb=avb):
                        k = hi * 8 + t
                        j, kk = divmod(k, 6)
                        return avb[j][:, kk * 65:(kk + 1) * 65]
                else:
                    av = avp.tile([128, 1024], F32, tag="av",
                                  name=f"av{p}_{th}")
                    zt = ztp.tile([128, 16], F32, tag="zt", name=f"zt{p}_{th}")
                units = [(s, hi) for s in range(NS) for hi in range(2)]
                sc_t, pt_t = {}, {}

                def emit_sc_exp(u, p=p, th=th, m=m, sc_t=sc_t, pt_t=pt_t):
                    s, hi = u
                    h = 2 * p + hi
                    qo = (h % 4) * 32
                    sca = scp.tile([128, 512], F32, tag="sca",
                                   name=f"sca{p}_{th}_{s}_{hi}")
                    scd = scp.tile([128, 512], F32, tag="scd",
                                   name=f"scd{p}_{th}_{s}_{hi}")
                    for nj, sc in ((0, sca), (1, scd)):
                        nc.tensor.matmul(
                            sc[:],
                            lhsT=kt[m][qo:qo + 32, s * 128:(s + 1) * 128],
                            rhs=qt[m][qo:qo + 32,
                                      th * 1024 + nj * 512:
                                      th * 1024 + (nj + 1) * 512],
                            start=True, stop=True, tile_position=(qo, 0))
                    pta = ptp.tile([128, ACOL], F16, tag="pta",
                                   name=f"pta{p}_{th}_{s}_{hi}")
                    ptd = ptp.tile([128, 1024 - ACOL], I16, tag="ptd",
                                   name=f"ptd{p}_{th}_{s}_{hi}")
                    nc.scalar.activation(pta[:], sca[:], AF.Exp)
                    nc.vector.tensor_scalar(
                        ptd[:], scd[:],
                        SCH_A, SCH_B, OP.mult, OP.add)
                    sc_t[u] = (sca, scd)
                    pt_t[u] = (pta, ptd)

                nsplit = ACOL // 128

                def pt_chunk(pt, t):
                    pta, ptd = pt
                    if t < nsplit:
                        return pta[:, t * 128:(t + 1) * 128]
                    return ptd[:, (t - nsplit) * 128:
                               (t - nsplit + 1) * 128].bitcast(F16)

                if MERGEZ:
                    def emit_av(u, p=p, th=th, sc_t=sc_t, pt_t=pt_t,
                                av_slot=av_slot):
                        s, hi = u
                        pt = pt_t.pop(u)
                        del sc_t[u]
                        for t in range(8):
                            k = hi * 8 + t
                            nc.tensor.matmul(
                                av_slot(hi, t),
                                lhsT=pt_chunk(pt, t),
                                rhs=vv[s][:, p * 65:(p + 1) * 65],
                                start=(s == 0 and k in (0, 6, 12)),
                                stop=(s == NS - 1 and k in (5, 11, 15)),
                                skip_group_check=True)
                else:
                    def emit_av(u, p=p, av=av, zt=zt, sc_t=sc_t, pt_t=pt_t):
                        s, hi = u
                        pt = pt_t.pop(u)
                        del sc_t[u]
                        for t in range(8):
                            nc.tensor.matmul(
                                av[:, hi * 512 + t * 64:hi * 512 + (t + 1) * 64],
                                lhsT=pt_chunk(pt, t),
                                rhs=vv[s][:, p * 64:(p + 1) * 64],
                                start=(s == 0 and t == 0),
                                stop=(s == NS - 1 and t == 7),
                                skip_group_check=True)
                        for t in range(8):
                            nc.tensor.matmul(
                                zt[:, hi * 8 + t:hi * 8 + t + 1],
                                lhsT=pt_chunk(pt, t),
                                rhs=ones16[:],
                                start=(s == 0 and hi == 0 and t == 0),
                                stop=(s == NS - 1 and hi == 1 and t == 7),
                                skip_group_check=True)

                emit_sc_exp(units[0])
                for i, u in enumerate(units):
                    if i + 1 < len(units):
                        emit_sc_exp(units[i + 1])
                    emit_av(u)
                    while pending and pending[0][0] <= i:
                        pending.pop(0)[1]()

                # ---- epilogue ----
                rec = epp.tile([128, 16], F32, tag="rec",
                               name=f"rec{p}_{th}")
                if MERGEZ:
                    # gather z cols (slot col 64, stride 65) from 3 bank tiles
                    nc.vector.reciprocal(
                        rec[:, 0:6], avb[0][:, 64:64 + 6 * 65:65])
                    nc.vector.reciprocal(
                        rec[:, 6:12], avb[1][:, 64:64 + 6 * 65:65])
                    nc.vector.reciprocal(
                        rec[:, 12:16], avb[2][:, 64:64 + 4 * 65:65])
                else:
                    nc.vector.reciprocal(rec[:], zt[:, 0:16])
                rec2l = epp.tile([128, 8], F32, tag="rec2l",
                                 name=f"rec2l{p}_{th}")
                nc.vector.tensor_scalar_mul(rec2l[:], rec[:, 8:16], -lam)

                tmp = epp.tile([128, 512], F32, tag="tmp",
                               name=f"tmp{p}_{th}")
                e2 = epp.tile([128, 512], F32, tag="e2", name=f"e2{p}_{th}")
                if MERGEZ:
                    # slots k=0..15 (65-wide, 6|6|4 per bank tile); o-part is
                    # a 3D strided view, scaled by the per-partition rec col.
                    def oview(j, a, n):
                        return avb[j][:, a * 65:(a + n) * 65].rearrange(
                            "p (t c) -> p t c", c=65)[:, :, 0:64]
                    def rview(r, a, n):
                        return r[:, a:a + n].unsqueeze(2).broadcast_to(
                            [128, n, 64])
                    t3 = tmp[:].rearrange("p (t c) -> p t c", c=64)
                    e3 = e2[:].rearrange("p (t c) -> p t c", c=64)
                    nc.vector.tensor_mul(t3[:, 0:6], oview(0, 0, 6),
                                         rview(rec, 0, 6))
                    nc.vector.tensor_mul(t3[:, 6:8], oview(1, 0, 2),
                                         rview(rec, 6, 2))
                    nc.vector.tensor_mul(e3[:, 0:4], oview(1, 2, 4),
                                         rview(rec2l, 0, 4))
                    nc.vector.tensor_mul(e3[:, 4:8], oview(2, 0, 4),
                                         rview(rec2l, 4, 4))
                else:
                    o1 = av[:, 0:512].rearrange("p (t c) -> p t c", c=64)
                    o2 = av[:, 512:1024].rearrange("p (t c) -> p t c", c=64)
                    recB = rec[:, 0:8].unsqueeze(2).broadcast_to([128, 8, 64])
                    rec2B = rec2l[:].unsqueeze(2).broadcast_to([128, 8, 64])
                    nc.vector.tensor_mul(
                        tmp[:].rearrange("p (t c) -> p t c", c=64), o1, recB)
                    nc.vector.tensor_mul(
                        e2[:].rearrange("p (t c) -> p t c", c=64), o2, rec2B)

                def late1(tmp=tmp, e2=e2, p=p, th=th):
                    attn = epp.tile([128, 512], F32, tag="attn",
                                    name=f"attn{p}_{th}")
                    nc.gpsimd.tensor_add(attn[:], tmp[:], e2[:])
                    sq = epp.tile([128, 512], F32, tag="sq",
                                  name=f"sq{p}_{th}")
                    nc.gpsimd.tensor_mul(sq[:], attn[:], attn[:])
                    ssq = epp.tile([128, 8], F32, tag="ssq",
                                   name=f"ssq{p}_{th}")
                    nc.vector.reduce_sum(
                        ssq[:], sq[:].rearrange("p (t c) -> p t c", c=64),
                        axis=mybir.AxisListType.X)
                    mm_ = epp.tile([128, 8], F32, tag="mm", name=f"mm{p}_{th}")
                    nc.vector.tensor_scalar(mm_[:], ssq[:], 1.0 / 64.0,
                                            EPS, OP.mult, OP.add)
                    it_ = epp.tile([128, 8], I32, tag="it", name=f"it{p}_{th}")
                    nc.vector.tensor_scalar(
                        it_[:], mm_[:].bitcast(I32), 1, None,
                        OP.logical_shift_right)
                    nc.vector.tensor_scalar(it_[:], it_[:], -1, None,
                                            OP.bitwise_xor)
                    nc.vector.tensor_scalar(it_[:], it_[:], 0x5f3759df + 1,
                                            None, OP.add)
                    y0 = it_[:].bitcast(F32)
                    nw = epp.tile([128, 8], F32, tag="nw", name=f"nw{p}_{th}")
                    nc.vector.tensor_mul(nw[:], y0, y0)
                    nc.vector.tensor_mul(nw[:], nw[:], mm_[:])
                    nc.vector.tensor_scalar(nw[:], nw[:], -0.5, 1.5,
                                            OP.mult, OP.add)
                    rinv = epp.tile([128, 8], F32, tag="rinv",
                                    name=f"rinv{p}_{th}")
                    nc.vector.tensor_mul(rinv[:], y0, nw[:])

                    a16 = epp.tile([128, 512], F16, tag="a16",
                                   name=f"a16_{p}_{th}")
                    rinvB = rinv[:].unsqueeze(2).broadcast_to([128, 8, 64])
                    nc.gpsimd.tensor_mul(
                        a16[:].rearrange("p (t c) -> p t c", c=64),
                        attn[:].rearrange("p (t c) -> p t c", c=64), rinvB)
                    return a16

                state = {}

                def late1_run(state=state, late1=late1):
                    state["a16"] = late1()

                def late2(state=state, row=row, c=p // 2, th=th, p=p):
                    a16 = state["a16"]
                    tp = tpp.tile([128, 1024], F16, tag="tp",
                                  name=f"tp{p}_{th}")
                    for t in range(8):
                        nc.tensor.matmul(
                            tp[row:row + 64, t * 128:(t + 1) * 128],
                            lhsT=a16[:, t * 64:(t + 1) * 64],
                            rhs=ident[:], is_transpose=True,
                            start=(t == 0), stop=(t == 7),
                            skip_group_check=True)
                    nc.scalar.copy(
                        at[c][row:row + 64, th * 1024:(th + 1) * 1024],
                        tp[row:row + 64, :])

                pending = [(4, late1_run), (10, late2)]

        for _, fn in pending:
            fn()
        pending = []

    # ---------------- Phase C: output projection ----------------
    with ExitStack() as cctx:
      if "C" in phases:
        cpool = cctx.enter_context(
            tc.tile_pool(name=f"cpsum{it}", bufs=8, space="PSUM"))
        spool = cctx.enter_context(tc.tile_pool(name=f"cst{it}", bufs=8))
        for e in range(8):
            for nt in range(4):
                ps = cpool.tile([128, 512], F32, tag="op",
                                name=f"ops{e}_{nt}")
                for c in range(2):
                    nc.tensor.matmul(
                        ps[:],
                        lhsT=wo_sb[c][:, e * 128:(e + 1) * 128],
                        rhs=at[c][:, nt * 512:(nt + 1) * 512],
                        start=(c == 0), stop=(c == 1))
                st = spool.tile([128, 512], F16, tag="st",
                                name=f"st{e}_{nt}")
                if (e * 4 + nt) % 2 == 0:
                    nc.scalar.copy(st[:], ps[:])
                else:
                    nc.vector.tensor_copy(st[:], ps[:])
                nc.sync.dma_start(
                    out=out_d[e * 128:(e + 1) * 128,
                              nt * 512:(nt + 1) * 512],
                    in_=st[:])


MERGEZ = True
PRETRANS = True


def _build(lam: float, phases: str = "ABC", unroll: int = 1):
    nc = bacc.Bacc("TRN2", target_bir_lowering=False, debug=False,
                   enable_asserts=False, num_devices=8)

    if PRETRANS:
        xq_d = nc.dram_tensor("xq_t", [EMBED, T], F16, kind="ExternalInput").ap()
        xkv_d = nc.dram_tensor("xkv_t", [EMBED, S], F16, kind="ExternalInput").ap()
    else:
        xq_d = nc.dram_tensor("xq_r", [T, EMBED], F16, kind="ExternalInput").ap()
        xkv_d = nc.dram_tensor("xkv_r", [S, EMBED], F16, kind="ExternalInput").ap()
    wq_d = nc.dram_tensor("wq", [EMBED, HC], F16, kind="ExternalInput").ap()
    wk_d = nc.dram_tensor("wk", [EMBED, HC], F16, kind="ExternalInput").ap()
    wv_d = nc.dram_tensor("wv", [EMBED, HC], F16, kind="ExternalInput").ap()
    wo_d = nc.dram_tensor("wo", [HC, EMBED], F16, kind="ExternalInput").ap()
    out_d = nc.dram_tensor("out_t", [EMBED, T], F16, kind="ExternalOutput").ap()
    dram = (xq_d, xkv_d, wq_d, wk_d, wv_d, wo_d, out_d)

    with tile.TileContext(nc) as tc, ExitStack() as ctx:
        for it in range(unroll):
            with ExitStack() as ictx:
                _emit(nc, tc, ictx, dram, lam, phases, it)

    nc.compile()
    return nc


def _lam_of(inputs):
    lam1 = np.exp(np.sum(np.asarray(inputs["lambda_q1"], np.float32)
                         * np.asarray(inputs["lambda_k1"], np.float32),
                         dtype=np.float32))
    lam2 = np.exp(np.sum(np.asarray(inputs["lambda_q2"], np.float32)
                         * np.asarray(inputs["lambda_k2"], np.float32),
                         dtype=np.float32))
    return float(np.float32(lam1 - lam2 + np.float32(LAMBDA_INIT)))


def _in_maps(inputs):
    q = np.asarray(inputs["query_x"], np.float32)
    kv = np.asarray(inputs["kv_x"], np.float32)
    Wq = np.asarray(inputs["Wq"], np.float32)
    Wk = np.asarray(inputs["Wk"], np.float32)
    Wv = np.asarray(inputs["Wv"], np.float32)
    Wo = np.asarray(inputs["Wo"], np.float32)
    subln_w = np.asarray(inputs["subln_w"], np.float32)

    scaling = np.float32(D ** -0.5)
    wo_scale = (np.tile(subln_w, PAIRS) * np.float32(1.0 - LAMBDA_INIT))
    if PRETRANS:
        xq16 = [np.ascontiguousarray(q[b].T).astype(np.float16)
                for b in range(2)]
        xkv16 = [np.ascontiguousarray(kv[b].T).astype(np.float16)
                 for b in range(2)]
        xq_key, xkv_key = "xq_t", "xkv_t"
    else:
        xq16 = [q[b].astype(np.float16) for b in range(2)]
        xkv16 = [kv[b].astype(np.float16) for b in range(2)]
        xq_key, xkv_key = "xq_r", "xkv_r"
    maps = []
    for core in range(8):
        b, g = divmod(core, 4)
        sl = slice(g * HC, (g + 1) * HC)
        maps.append({
            xq_key: xq16[b],
            xkv_key: xkv16[b],
            "wq": (Wq[:, sl] * scaling).astype(np.float16),
            "wk": Wk[:, sl].astype(np.float16),
            "wv": Wv[:, sl].astype(np.float16),
            "wo": (Wo[sl, :] * wo_scale[:, None]).astype(np.float16),
        })
    return maps


def _get_nc(inputs, phases="ABC", unroll=1):
    lam = _lam_of(inputs)
    key = (round(lam, 12), phases, unroll, MERGEZ, PRETRANS)
    if key not in _cache:
        _cache[key] = _build(lam, phases, unroll)
    return _cache[key]


def _run(inputs):
    nc = _get_nc(inputs)
    in_maps = _in_maps(inputs)
    res = run_bass_kernel_spmd(nc, in_maps, list(range(8)), trace=False)
    out = np.zeros((2, T, EMBED), np.float32)
    for core in range(8):
        out[core // 4] += res.results[core]["out_t"].T.astype(np.float32)
    return out, res


def kernel(**inputs):
    out, _ = _run(inputs)
    return out


def _run_timed(inputs, iters=30, phases="ABC", unroll=64):
    """Times an unroll-N NEFF with device-resident inputs and reports the
    steady-state marginal per-call slope divided by N. The slope between two
    batch sizes cancels the fixed client/axon dispatch overhead; dividing by
    the in-NEFF unroll amortizes the ~0.5ms per-call RPC floor so the number
    approximates true per-kernel device execution time."""
    import time
    import jax
    from jax.experimental.shard_map import shard_map
    from jax.sharding import Mesh, NamedSharding, PartitionSpec
    from concourse import bass2jax, mybir as mb

    nc = _get_nc(inputs, phases, unroll)
    in_maps = _in_maps(inputs)

    bass2jax.install_neuronx_cc_hook()
    n_cores = 8
    partition_name = (nc.partition_id_tensor.name
                      if nc.partition_id_tensor else None)
    in_names, out_names, out_avals, zero_outs = [], [], [], []
    for alloc in nc.m.functions[0].allocations:
        if not isinstance(alloc, mb.MemoryLocationSet):
            continue
        name = alloc.memorylocations[0].name
        if alloc.kind == "ExternalInput":
            if name != partition_name:
                in_names.append(name)
        elif alloc.kind == "ExternalOutput":
            out_names.append(name)
            shape = tuple(alloc.tensor_shape)
            dtype = mb.dt.np(alloc.dtype)
            out_avals.append(jax.core.ShapedArray(shape, dtype))
            zero_outs.append(np.zeros(shape, dtype))
    n_params = len(in_names)
    all_names = in_names + out_names
    if partition_name is not None:
        all_names = all_names + [partition_name]

    def _body(*args):
        operands = list(args)
        if partition_name is not None:
            operands.append(bass2jax.partition_id_tensor())
        outs = bass2jax._bass_exec_p.bind(
            *operands,
            out_avals=tuple(out_avals),
            in_names=tuple(all_names),
            out_names=tuple(out_names),
            lowering_input_output_aliases=(),
            sim_require_finite=True,
            sim_require_nnan=True,
            nc=nc,
        )
        return tuple(outs)

    devices = jax.devices()[:n_cores]
    mesh = Mesh(np.asarray(devices), ("core",))
    spec = NamedSharding(mesh, PartitionSpec("core"))
    n_outs = len(out_names)
    sharded = jax.jit(
        shard_map(_body, mesh=mesh,
                  in_specs=(PartitionSpec("core"),) * (n_params + n_outs),
                  out_specs=(PartitionSpec("core"),) * n_outs,
                  check_rep=False),
        keep_unused=True)

    concat_in = [
        jax.device_put(
            np.concatenate([in_maps[c][nm] for c in range(n_cores)], axis=0),
            spec)
        for nm in in_names
    ]
    concat_zeros = [
        jax.device_put(np.zeros((n_cores * z.shape[0], *z.shape[1:]), z.dtype),
                       spec)
        for z in zero_outs
    ]

    out_arrs = sharded(*concat_in, *concat_zeros)  # compile + first run
    jax.block_until_ready(out_arrs)
    for _ in range(4):  # warmup
        jax.block_until_ready(sharded(*concat_in, *concat_zeros))

    lo, hi = max(20, iters // 3), max(60, iters * 2)
    best = {}
    for n in (lo, hi):
        b = None
        for _rep in range(6):
            t0 = time.perf_counter()
            rs = [sharded(*concat_in, *concat_zeros) for _ in range(n)]
            jax.block_until_ready(rs)
            t1 = time.perf_counter()
            tt = t1 - t0
            b = tt if b is None else min(b, tt)
        best[n] = b
    marginal_per_call = (best[hi] - best[lo]) / (hi - lo)
    per_kernel_ns = marginal_per_call / unroll * 1e9

    out = np.zeros((2, T, EMBED), np.float32)
    full = np.asarray(out_arrs[0]).reshape(n_cores, EMBED, T)
    for core in range(8):
        out[core // 4] += full[core].T.astype(np.float32)
    return out, per_kernel_ns
